# revision 1
# baseline (speedup 1.0000x reference)
"""Additive attention (Bahdanau) Trainium2 kernel, SPMD across 8 NeuronCores.

Reference computation (per batch b):
    q = Q[b] @ Wq                 [NQ, H]
    k = K[b] @ Wk                 [NK, H]
    scores[i, j] = sum_h Wv[h] * tanh(q[i, h] + k[j, h])
    attn = softmax(mask(scores))  (keys >= valid_len[b] masked to -1e6)
    out[b] = attn @ V[b]

Sharding: core c handles queries [c*QG, (c+1)*QG) of EVERY batch (QG =
NQ/8).  Each batch's key range is truncated to its valid_len at
graph-build time (valid_lens are host-visible), so no masking is ever
needed: keys beyond valid_len contribute exactly 0 to the reference
softmax (exp(-1e6) underflows to 0), so truncation is exact.  Softmax is
computed without max-subtraction: |scores| <= sum|Wv| ~ 10, safely inside
f32/bf16 exp range, and the reference ratio is identical.

Engine mapping (per core, per batch-group g of QG queries x vl_g keys):
  PE   : qT/kT projections; per-(query, key-block) score matmuls
         (lhsT = tanh tile [H, keys], rhs = Wv [H, 1]) which produce the
         scores TRANSPOSED [keys, query] - exactly the attn @ V layout;
         softmax-denominator ones-matmuls; attn @ V matmuls.
  DVE  : per-query broadcast adds q[:,qi] + kT (tensor_scalar, bf16 4x);
         projection PSUM->SBUF copies; reciprocal; final 1/Z scaling.
  ACT  : big-tile tanh (bf16); exp of the transposed scores.
  SYNC : all DMA (HWDGE).
All reductions accumulate in f32 PSUM; bf16 only on storage/stream paths.
"""

import math

import numpy as np
import ml_dtypes

import concourse.bass as bass
import concourse.mybir as mybir
from concourse.bass_utils import run_bass_kernel_spmd

BF16 = mybir.dt.bfloat16
F32 = mybir.dt.float32
AF = mybir.ActivationFunctionType

N_CORES = 8


def build_graph(vls, B=4, H=128, DQ=512, DK=512, DV=512, QG=64, QC=16, iters=1,
                bench=(), tanh_f32=False, abias=False, nbufs=2, nsc=2,
                elate=False, epl=False, gorder=False, tmerge=False):
    """Build the per-core bass graph. vls: per-batch valid lens (python ints).

    iters > 1 unrolls the whole group pipeline iters times back-to-back in
    one NEFF (same data), for marginal-cost timing immune to dispatch
    overhead.  Group indices gg run over iters*B; batch identity is gg %% B.
    """
    assert H == 128 and DQ % 128 == 0 and DK % 128 == 0
    NCH = QG // QC
    assert NCH * QC == QG and NCH % 2 == 0
    GG = iters * B
    if gorder:
        order = sorted(range(B), key=lambda b: vls[b])
        gmap = [order[0]] + sorted(order[1:], key=lambda b: -vls[b])
    else:
        gmap = list(range(B))
    W = [int(v + (v & 1)) for v in vls]          # even widths (DVE 4x mode)
    offs = [0]
    for w in W:
        offs.append(offs[-1] + w)
    Wsum = offs[-1]
    Wmax = max(W)
    nb = [max(1, math.ceil(v / 128)) for v in vls]  # key-blocks per group
    nbmax = max(nb)
    vco = [0]
    for n in nb:
        vco.append(vco[-1] + n)
    NVC = vco[-1]
    nDQ, nDK = DQ // 128, DK // 128

    nc = bass.Bass()

    qT_e = nc.declare_dram_parameter("qT", [128, nDQ, B, QG], BF16, isOutput=False)
    kT_e = nc.declare_dram_parameter("kT", [128, nDK, Wsum], BF16, isOutput=False)
    v_e = nc.declare_dram_parameter("v", [128, NVC, DV], BF16, isOutput=False)
    wq_e = nc.declare_dram_parameter("wq", [128, nDQ, H], BF16, isOutput=False)
    wk_e = nc.declare_dram_parameter("wk", [128, nDK, H], BF16, isOutput=False)
    wv_dt = F32 if tanh_f32 else BF16
    wv_e = nc.declare_dram_parameter("wv", [H, 1], wv_dt, isOutput=False)
    ones_e = nc.declare_dram_parameter("ones", [128, 1], BF16, isOutput=False)
    out_e = nc.declare_dram_parameter("out", [B, QG, DV], F32, isOutput=True)

    # ---- pass A: enumerate semaphore-inc orders per engine -----------------
    class S:
        def __init__(self):
            self.n = 0
            self.idx = {}

        def inc(self, tag):
            self.n += 1
            if tag is not None:
                self.idx[tag] = self.n
            return self.n

    dma, pe, act, dve = S(), S(), S(), S()

    LOADS = ["wq", "wk", "qT", "wv", "ones"] + [
        f"kT{g}" for g in range(B)] + [f"v{g}" for g in range(B)]

    # pe order: 8 proj groups (q0,k0,q1,k1,...), then per-group score-matmul
    # chunks, with z(g-1)+vm(g-1) interleaved right after chunk (g, 0).
    E_C = 1 if elate else 0
    for i in range(2 * B):
        pe.inc(("proj", i))
    for gg in range(GG):
        for c in range(NCH):
            pe.inc(("m", gg, c))
            if c == E_C and gg >= 1:
                pe.inc(("z", gg - 1))
                pe.inc(("vm", gg - 1))
    pe.inc(("z", GG - 1))
    pe.inc(("vm", GG - 1))

    # act order: tanh chunks; exp(g) deferred until after tanh(g+1, 0)
    act_order = []
    if tmerge:
        assert NCH % 2 == 0 and nbufs % 2 == 0
        for gg in range(GG):
            for c in range(0, NCH, 2):
                act_order.append(("t", gg, c))
                if gg >= 1 and c == (E_C // 2) * 2:
                    act_order.append(("e", gg - 1))
        act_order.append(("e", GG - 1))
        for tag in act_order:
            act.inc(tag)
            if tag[0] == "t":
                act.idx[("t", tag[1], tag[2] + 1)] = act.idx[tag]
    else:
        for gg in range(GG):
            for c in range(NCH):
                act_order.append(("t", gg, c))
                if gg >= 1 and c == E_C:
                    act_order.append(("e", gg - 1))
        act_order.append(("e", GG - 1))
        for tag in act_order:
            act.inc(tag)

    # dve order: memset, proj copies, adds with epilogue(g-1) interleaved
    dve.inc(("ms",))
    dve.inc(("msc0",))
    dve.inc(("msc1",))
    for i in range(2 * B):
        dve.inc(("pc", i))
    C_R, C_O = (NCH - 1, NCH - 1) if epl else (min(1, NCH - 1), min(2, NCH - 1))
    for gg in range(GG):
        for c in range(NCH):
            if not abias:
                dve.inc(("a", gg, c))
            if c == C_R and gg >= 1:
                dve.inc(("r", gg - 1))
            if c == C_O and gg >= 1:
                dve.inc(("o", gg - 1))
    dve.inc(("r", GG - 1))
    dve.inc(("o", GG - 1))

    def chunk_gc(tau):
        # global chunk index -> (gg, c)
        return tau // NCH, tau % NCH

    # ---- allocate memory + emit ------------------------------------------
    from contextlib import ExitStack

    es = ExitStack()
    with es:
        wq_sb = es.enter_context(nc.sbuf_tensor([128, nDQ, H], BF16))
        wk_sb = es.enter_context(nc.sbuf_tensor([128, nDK, H], BF16))
        qT_sb = es.enter_context(nc.sbuf_tensor([128, nDQ, B, QG], BF16))
        kT_sb = es.enter_context(nc.sbuf_tensor([128, nDK, Wsum], BF16))
        v_sb = es.enter_context(nc.sbuf_tensor([128, NVC, DV], BF16))
        wv_sb = es.enter_context(nc.sbuf_tensor([128, 1], wv_dt))
        ones_sb = es.enter_context(nc.sbuf_tensor([128, 1], BF16))
        qTp_sb = es.enter_context(nc.sbuf_tensor([128, B, QG], F32))
        kTp_sb = es.enter_context(nc.sbuf_tensor([128, Wsum], BF16))
        sum_sb = es.enter_context(nc.sbuf_tensor([128, nbufs, QC * Wmax], BF16))
        tanh_sb = es.enter_context(
            nc.sbuf_tensor([128, nbufs, QC * Wmax], F32 if tanh_f32 else BF16)
        )
        exp_sb = es.enter_context(nc.sbuf_tensor([128, nsc, nbmax, QG], BF16))
        recip_sb = es.enter_context(nc.sbuf_tensor([QG, B], F32))
        out_sb = es.enter_context(nc.sbuf_tensor([QG, 2, DV], F32))
        scratch = es.enter_context(nc.sbuf_tensor([1, 8], F32))
        scratch2 = es.enter_context(nc.sbuf_tensor([1, 8], F32))

        pp = [
            es.enter_context(nc.psum_tensor(f"pp{i}", [128, 512], F32))
            for i in range(2)
        ]
        sc = [
            es.enter_context(nc.psum_tensor(f"sc{i}", [128, nbmax, QG], F32))
            for i in range(nsc)
        ]
        op = [
            es.enter_context(nc.psum_tensor(f"op{i}", [QG, DV], F32))
            for i in range(2)
        ]
        z_ps = es.enter_context(nc.psum_tensor("z_ps", [QG, B], F32))

        ld_sem = {
            name: es.enter_context(nc.semaphore(f"ld_{name}")) for name in LOADS
        }
        ost_sem = [
            es.enter_context(nc.semaphore(f"ost{i}")) for i in range(2)
        ]
        pe_sem = es.enter_context(nc.semaphore("pe_sem"))
        act_sem = es.enter_context(nc.semaphore("act_sem"))
        dve_sem = es.enter_context(nc.semaphore("dve_sem"))
        block = es.enter_context(nc.Block())

        @block.sync
        def _(sy):
            sy.dma_start(out=wq_sb[:], in_=wq_e[:]).then_inc(ld_sem["wq"], 16)
            sy.dma_start(out=qT_sb[:], in_=qT_e[:]).then_inc(ld_sem["qT"], 16)
            sy.dma_start(out=wk_sb[:], in_=wk_e[:]).then_inc(ld_sem["wk"], 16)
            g0 = gmap[0]
            sy.dma_start(
                out=kT_sb[:, :, offs[g0] : offs[g0] + W[g0]],
                in_=kT_e[:, :, offs[g0] : offs[g0] + W[g0]],
            ).then_inc(ld_sem[f"kT{g0}"], 16)
            sy.dma_start(out=wv_sb[:], in_=wv_e[:]).then_inc(ld_sem["wv"], 16)
            sy.dma_start(out=ones_sb[:], in_=ones_e[:]).then_inc(ld_sem["ones"], 16)
            for g in gmap[1:]:
                sy.dma_start(
                    out=kT_sb[:, :, offs[g] : offs[g] + W[g]],
                    in_=kT_e[:, :, offs[g] : offs[g] + W[g]],
                ).then_inc(ld_sem[f"kT{g}"], 16)
            for g in gmap:
                sy.dma_start(
                    out=v_sb[:, vco[g] : vco[g] + nb[g], :],
                    in_=v_e[:, vco[g] : vco[g] + nb[g], :],
                ).then_inc(ld_sem[f"v{g}"], 16)
            for gg in range(GG):
                sy.wait_ge(dve_sem, dve.idx[("o", gg)])
                sy.dma_start(
                    out=out_e[gmap[gg % B]], in_=out_sb[0:QG, gg % 2, :]
                ).then_inc(ost_sem[gg % 2], 16)

        @block.tensor
        def _(pe_eng):
            def proj(i):
                is_q, gsl = (i % 2 == 0), i // 2
                g = gmap[gsl]
                if i == 0:
                    pe_eng.wait_ge(ld_sem["wq"], 16)
                    pe_eng.wait_ge(ld_sem["qT"], 16)
                if i == 1:
                    pe_eng.wait_ge(ld_sem["wk"], 16)
                if not is_q:
                    pe_eng.wait_ge(ld_sem[f"kT{g}"], 16)
                if i >= 2:
                    pe_eng.wait_ge(dve_sem, dve.idx[("pc", i - 2)])
                nch = nDQ if is_q else nDK
                for c in range(nch):
                    if is_q:
                        o, l, r = (
                            pp[i % 2][0:128, 0:QG],
                            wq_sb[:, c, :],
                            qT_sb[:, c, g, :],
                        )
                    else:
                        o, l, r = (
                            pp[i % 2][0:128, 0 : W[g]],
                            wk_sb[:, c, :],
                            kT_sb[:, c, offs[g] : offs[g] + W[g]],
                        )
                    mm = pe_eng.matmul(o, l, r, start=(c == 0), stop=(c == nch - 1))
                mm.then_inc(pe_sem, 1)

            for i in range(2 * B):
                proj(i)

            pe_eng.wait_ge(ld_sem["wv"], 16)

            def mm_chunk(gg, c):
                g = gmap[gg % B]
                tau = gg * NCH + c
                pe_eng.wait_ge(act_sem, act.idx[("t", gg, c)])
                for qq in range(1 if "mm" in bench else QC):
                    qi = c * QC + qq
                    for b in range(nb[g]):
                        sz = min(128, vls[g] - 128 * b)
                        mm = pe_eng.matmul(
                            sc[gg % nsc][0:sz, b, qi : qi + 1],
                            tanh_sb[
                                :, tau % nbufs,
                                qq * W[g] + 128 * b : qq * W[g] + 128 * b + sz
                            ],
                            wv_sb[:, 0:1],
                            start=True,
                            stop=True,
                        )
                mm.then_inc(pe_sem, 1)

            def zmm(gg):
                g = gmap[gg % B]
                pe_eng.wait_ge(act_sem, act.idx[("e", gg)])
                if gg == 0:
                    pe_eng.wait_ge(ld_sem["ones"], 16)
                for b in range(nb[g]):
                    sz = min(128, vls[g] - 128 * b)
                    mm = pe_eng.matmul(
                        z_ps[0:QG, g : g + 1],
                        exp_sb[0:sz, gg % nsc, b, :],
                        ones_sb[0:sz, :],
                        start=(b == 0),
                        stop=(b == nb[g] - 1),
                    )
                mm.then_inc(pe_sem, 1)

            def vmm(gg):
                g = gmap[gg % B]
                if gg < B:
                    pe_eng.wait_ge(ld_sem[f"v{g}"], 16)
                for b in range(nb[g]):
                    sz = min(128, vls[g] - 128 * b)
                    mm = pe_eng.matmul(
                        op[gg % 2][0:QG, 0:DV],
                        exp_sb[0:sz, gg % nsc, b, :],
                        v_sb[0:sz, vco[g] + b, :],
                        start=(b == 0),
                        stop=(b == nb[g] - 1),
                    )
                mm.then_inc(pe_sem, 1)

            for gg in range(GG):
                for c in range(NCH):
                    mm_chunk(gg, c)
                    if c == E_C and gg >= 1:
                        zmm(gg - 1)
                        vmm(gg - 1)
            zmm(GG - 1)
            vmm(GG - 1)

        @block.scalar
        def _(sa):
            sa.wait_ge(dve_sem, dve.idx[("ms",)])
            sa.activation(scratch2[0:1, 0:2], scratch[0:1, 0:2], AF.Tanh)

            def tanh_op(gg, c):
                g = gmap[gg % B]
                tau = gg * NCH + c
                if tmerge:
                    sa.wait_ge(dve_sem, dve.idx[("a", gg, c + 1)])
                    if tau + 1 >= nbufs:
                        g2, c2 = chunk_gc(tau + 1 - nbufs)
                        sa.wait_ge(pe_sem, pe.idx[("m", g2, c2)])
                    s = tau % nbufs
                    tw = 64 if "tanh" in bench else QC * W[g]
                    sa.activation(
                        tanh_sb[:, s : s + 2, 0:tw],
                        sum_sb[:, s : s + 2, 0:tw],
                        AF.Tanh,
                    ).then_inc(act_sem, 1)
                    return
                if not abias:
                    sa.wait_ge(dve_sem, dve.idx[("a", gg, c)])
                elif c == 0 and gg < B:
                    sa.wait_ge(dve_sem, dve.idx[("pc", 2 * (gg % B) + 1)])
                if tau >= nbufs:
                    g2, c2 = chunk_gc(tau - nbufs)
                    sa.wait_ge(pe_sem, pe.idx[("m", g2, c2)])
                if abias:
                    for qq in range(QC):
                        qi = c * QC + qq
                        tw = 64 if "tanh" in bench else W[g]
                        a = sa.activation(
                            tanh_sb[:, tau % nbufs, qq * W[g] : qq * W[g] + tw],
                            kTp_sb[:, offs[g] : offs[g] + tw],
                            AF.Tanh,
                            bias=qTp_sb[:, g, qi : qi + 1],
                        )
                    a.then_inc(act_sem, 1)
                else:
                    tw = 64 if "tanh" in bench else QC * W[g]
                    sa.activation(
                        tanh_sb[:, tau % nbufs, 0:tw],
                        sum_sb[:, tau % nbufs, 0:tw],
                        AF.Tanh,
                    ).then_inc(act_sem, 1)

            def exp_op(gg):
                g = gmap[gg % B]
                sa.wait_ge(pe_sem, pe.idx[("m", gg, NCH - 1)])
                sa.activation(
                    exp_sb[0:128, gg % nsc, 0 : nb[g], :],
                    sc[gg % nsc][0:128, 0 : nb[g], :],
                    AF.Exp,
                ).then_inc(act_sem, 1)

            for tag in act_order:
                if tag[0] == "t":
                    tanh_op(tag[1], tag[2])
                else:
                    exp_op(tag[1])

        @block.vector
        def _(ve):
            ve.memset(scratch[0:1, 0:8], 0.0).then_inc(dve_sem, 1)
            ve.memset(sc[0][:], 0.0).then_inc(dve_sem, 1)
            ve.memset(sc[1 % nsc][:], 0.0).then_inc(dve_sem, 1)
            for i in range(2, nsc):
                ve.memset(sc[i][:], 0.0)

            def proj_copy(i):
                is_q, g = (i % 2 == 0), gmap[i // 2]
                ve.wait_ge(pe_sem, pe.idx[("proj", i)])
                if is_q:
                    cp = ve.tensor_copy(qTp_sb[:, g, :], pp[i % 2][0:128, 0:QG])
                else:
                    cp = ve.tensor_copy(
                        kTp_sb[:, offs[g] : offs[g] + W[g]],
                        pp[i % 2][0:128, 0 : W[g]],
                    )
                cp.then_inc(dve_sem, 1)

            for i in range(2 * B):
                proj_copy(i)

            def adds(gg, c):
                g = gmap[gg % B]
                tau = gg * NCH + c
                if c == 0 and gg < B:
                    # scalar-ptr operands are prefetched at issue: wait for our
                    # own q-projection copy's sem inc before reading qTp scalars
                    ve.wait_ge(dve_sem, dve.idx[("pc", 2 * (gg % B))])
                if tau >= nbufs:
                    g2, c2 = chunk_gc(tau - nbufs)
                    ve.wait_ge(act_sem, act.idx[("t", g2, c2)])
                for qq in range(1 if "add" in bench else QC):
                    qi = c * QC + qq
                    a = ve.tensor_scalar_add(
                        sum_sb[:, tau % nbufs, qq * W[g] : (qq + 1) * W[g]],
                        kTp_sb[:, offs[g] : offs[g] + W[g]],
                        qTp_sb[:, g, qi : qi + 1],
                    )
                a.then_inc(dve_sem, 1)

            def ep_recip(gg):
                g = gmap[gg % B]
                ve.wait_ge(pe_sem, pe.idx[("z", gg)])
                ve.reciprocal(
                    recip_sb[0:QG, g : g + 1], z_ps[0:QG, g : g + 1]
                ).then_inc(dve_sem, 1)

            def ep_oscale(gg):
                g = gmap[gg % B]
                ve.wait_ge(dve_sem, dve.idx[("r", gg)])  # recip scalar-ptr hazard
                ve.wait_ge(pe_sem, pe.idx[("vm", gg)])
                if gg >= 2:
                    ve.wait_ge(ost_sem[gg % 2], 16 * (gg // 2))
                ve.tensor_scalar_mul(
                    out_sb[0:QG, gg % 2, :],
                    op[gg % 2][0:QG, 0:DV],
                    recip_sb[0:QG, g : g + 1],
                ).then_inc(dve_sem, 1)

            for gg in range(GG):
                for c in range(NCH):
                    if not abias:
                        adds(gg, c)
                    if c == C_R and gg >= 1:
                        ep_recip(gg - 1)
                    if c == C_O and gg >= 1:
                        ep_oscale(gg - 1)
            ep_recip(GG - 1)
            ep_oscale(GG - 1)

    return nc


def _host_prep(queries, keys, values, Wq, Wk, Wv, valid_lens,
               B, H, DQ, DK, DV, QG, QC, tanh_f32=False):
    bf = ml_dtypes.bfloat16
    vls = [int(v) for v in np.asarray(valid_lens)]
    W = [int(v + (v & 1)) for v in vls]
    offs = [0]
    for w in W:
        offs.append(offs[-1] + w)
    Wsum = offs[-1]
    nb = [max(1, math.ceil(v / 128)) for v in vls]
    vco = [0]
    for n in nb:
        vco.append(vco[-1] + n)
    NVC = vco[-1]

    nDQ, nDK = DQ // 128, DK // 128
    kT = np.zeros((DK, Wsum), np.float32)
    for b in range(B):
        kb = np.asarray(keys[b][: vls[b]]).T  # [DK, vl]
        kT[:, offs[b] : offs[b] + vls[b]] = kb
        if W[b] > vls[b]:
            kT[:, offs[b] + vls[b]] = kb[:, -1]
    kT = kT.reshape(nDK, 128, Wsum).transpose(1, 0, 2)  # [128, nDK, Wsum]
    v = np.zeros((128 * NVC, DV), np.float32)
    for b in range(B):
        v[128 * vco[b] : 128 * vco[b] + vls[b]] = values[b][: vls[b]]
    v = v.reshape(NVC, 128, DV).transpose(1, 0, 2)  # [128, NVC, DV]
    wq = np.asarray(Wq).reshape(nDQ, 128, H).transpose(1, 0, 2)
    wk = np.asarray(Wk).reshape(nDK, 128, H).transpose(1, 0, 2)
    # [128, nDQ, B, NQ]
    qT_full = np.asarray(queries).transpose(0, 2, 1).reshape(B, nDQ, 128, -1)
    qT_full = qT_full.transpose(2, 1, 0, 3)

    common = {
        "kT": np.ascontiguousarray(kT).astype(bf),
        "v": np.ascontiguousarray(v).astype(bf),
        "wq": np.ascontiguousarray(wq).astype(bf),
        "wk": np.ascontiguousarray(wk).astype(bf),
        "wv": np.ascontiguousarray(np.asarray(Wv).reshape(H, 1)).astype(
            np.float32 if tanh_f32 else bf),
        "ones": np.ones((128, 1), dtype=bf),
    }
    in_maps = []
    for c in range(N_CORES):
        m = dict(common)
        m["qT"] = np.ascontiguousarray(
            qT_full[:, :, :, c * QG : (c + 1) * QG]
        ).astype(bf)
        in_maps.append(m)
    return vls, in_maps


def kernel(queries, keys, values, Wq, Wk, Wv, valid_lens):
    B, NQ, DQ = queries.shape
    _, NK, DK = keys.shape
    DV = values.shape[2]
    H = Wq.shape[1]
    QG = NQ // N_CORES
    QC = 16 if QG % 16 == 0 else (8 if QG % 8 == 0 else QG)

    vls, in_maps = _host_prep(
        queries, keys, values, Wq, Wk, Wv, valid_lens, B, H, DQ, DK, DV, QG, QC
    )
    nc = build_graph(vls, B=B, H=H, DQ=DQ, DK=DK, DV=DV, QG=QG, QC=QC,
                     nbufs=3, epl=True, gorder=True)
    r = run_bass_kernel_spmd(nc, in_maps, core_ids=list(range(N_CORES)))
    out = np.empty((B, NQ, DV), np.float32)
    for c in range(N_CORES):
        out[:, c * QG : (c + 1) * QG, :] = r.results[c]["out"]
    return out



# revision 8
# speedup vs baseline: 1.6096x; 1.6096x over previous
"""Additive attention (Bahdanau) Trainium2 kernel, SPMD across 8 NeuronCores.

Reference computation (per batch b):
    q = Q[b] @ Wq                 [NQ, H]
    k = K[b] @ Wk                 [NK, H]
    scores[i, j] = sum_h Wv[h] * tanh(q[i, h] + k[j, h])
    attn = softmax(mask(scores))  (keys >= valid_len[b] masked to -1e6)
    out[b] = attn @ V[b]

KEY ALGORITHMIC CHANGE vs the tanh-materializing baseline: tanh(q+k) is a
smooth bivariate function of two ~N(0,1) scalars, so it admits a separable
(low-rank) approximation

    tanh(q + k) ~= sum_t c_t * d_{s(t)}(q) * K_{j(t)}(k)

with q-side dictionary {q^a * tanh(q)^e} and k-side menu
{z, z^2, z^3, t, t*z, t*z^2, t*z^3} (t = tanh(z)).  The structure (26
terms) is fixed; coefficients are re-fit at kernel() time by weighted
least squares on a Gauss-Hermite grid matched to the input scales.  The
[NQ, NK, H] intermediate is never materialized: scores become 26
PSUM-accumulated 128-contraction matmuls per (batch, key-block), with the
per-term coefficient and the Wv reduction folded into tiny [128,1]
per-partition scalars applied on the q side (tensor_scalar, DVE 4x mode).

Sharding: core c handles queries [c*QG, (c+1)*QG) of EVERY batch (QG =
NQ/8).  Each batch's key range is truncated to its valid_len at
graph-build time (valid_lens host-visible), so no masking is needed.
Softmax without max-subtraction (|scores| <~ 15, exp safe in f32).

Engine mapping (per core):
  PE   : q/k projections; 26-term score matmuls (full 128x128 array
         utilization); softmax-denominator ones-matmuls; attn @ V.
  ACT  : k-menu base tiles straight from the projection PSUM (Copy,
         Square, Tanh -- all in one act table with Exp: no table reload),
         q-side tanh, exp.
  DVE  : PSUM->SBUF q copies; dict/menu products (tensor_tensor, bf16
         2x); 26 scaled rhs copies (tensor_scalar, bf16 4x); reciprocal;
         1/Z output scaling.
  SYNC : all DMA.
"""

import math

import numpy as np
import ml_dtypes

import concourse.bass as bass
import concourse.mybir as mybir
from concourse.bass_utils import run_bass_kernel_spmd

BF16 = mybir.dt.bfloat16
F32 = mybir.dt.float32
AF = mybir.ActivationFunctionType

N_CORES = 8

# ---------------------------------------------------------------------------
# Approximation structure (fixed): backward-eliminated from the dense
# bilinear fit of tanh(sq*zq + sk*zk) over the product Gauss measure.
# q-dict keys: (a, e) -> zq^a * tanh(zq)^e ; k-menu names below.
KT_ORDER = ["z", "z2", "t", "z3", "tz", "tz2", "tz3"]
QD_ORDER = [(0, 0), (1, 0), (2, 0), (3, 0), (0, 1), (1, 1), (2, 1), (3, 1)]
SEL = [
    ((0, 0), "z"), ((0, 0), "z3"), ((0, 0), "tz2"),
    ((1, 0), "z2"), ((1, 0), "tz"), ((1, 0), "tz3"),
    ((2, 0), "z"), ((2, 0), "z3"), ((2, 0), "t"), ((2, 0), "tz2"),
    ((3, 0), "z2"), ((3, 0), "tz"),
    ((0, 1), "z2"), ((0, 1), "tz"), ((0, 1), "tz3"),
    ((1, 1), "z"), ((1, 1), "z3"), ((1, 1), "t"), ((1, 1), "tz2"),
    ((2, 1), "z2"), ((2, 1), "tz"), ((2, 1), "tz3"),
    ((3, 1), "z"), ((3, 1), "z3"), ((3, 1), "t"), ((3, 1), "tz2"),
]
NT = len(SEL)
# term emission order: grouped by k-tile build order so the PE can stream
KJ = {n: i for i, n in enumerate(KT_ORDER)}
TERMS = sorted(range(NT), key=lambda t: KJ[SEL[t][1]])  # term ids by tile


def _kfun(name, zk, tk):
    return {"z": zk, "z2": zk**2, "z3": zk**3, "t": tk,
            "tz": tk * zk, "tz2": tk * zk**2, "tz3": tk * zk**3,
            "c": np.ones_like(zk)}[name]


def fit_coefs(sq, sk, n=120):
    """Weighted LSQ coefficients for the fixed SEL structure, plus free
    constant-in-k pairs (softmax-invariant, dropped from the kernel).

    Ridge-regularized: the bf16 tiles round at ~0.4% relative, so a
    coefficient c on a term with L2(gauss) column norm ||A_t|| injects
    ~EPS*|c|*||A_t|| of incoherent score noise.  Choosing lambda to
    minimize  residual^2 + (EPS*||D c||)^2  trades fit error against
    bf16 noise amplification directly."""
    EPS = 0.004
    xs, wx = np.polynomial.hermite_e.hermegauss(n)
    wx = wx / wx.sum()
    zq = xs
    zk = xs
    F = np.tanh(sq * zq[:, None] + sk * zk[None, :])
    sw = np.sqrt(np.outer(wx, wx))
    tgt = (F * sw).ravel()
    tq = np.tanh(zq)
    tk = np.tanh(zk)
    qd = {(a, e): zq**a * tq**e for (a, e) in QD_ORDER}
    cols = []
    for (qk, kn) in SEL:
        cols.append((np.outer(qd[qk], _kfun(kn, zk, tk)) * sw).ravel())
    for qk in QD_ORDER:  # constant-in-k: free via softmax invariance
        cols.append((np.outer(qd[qk], _kfun("c", zk, tk)) * sw).ravel())
    A = np.stack(cols, axis=1)
    d = np.linalg.norm(A, axis=0)
    d[NT:] *= 1e-3  # const-pairs are noise-free (not emitted): barely penalize
    lam = 0.35     # empirically optimal vs bf16-simulated end-to-end error
    Ar = np.concatenate([A, lam * EPS * np.diag(d)], axis=0)
    br = np.concatenate([tgt, np.zeros(len(d))])
    coef, *_ = np.linalg.lstsq(Ar, br, rcond=None)
    return coef[:NT]


# ---------------------------------------------------------------------------
def build_graph2(vls, B=4, H=128, DQ=512, DK=512, DV=512, QG=64, iters=1,
                 debug=False):
    """Per-core bass graph. vls: per-batch valid lens (python ints).
    iters > 1 unrolls the whole per-core computation (everything except
    the one-time input loads) for marginal-cost timing."""
    assert H == 128 and DQ % 128 == 0 and DK % 128 == 0
    W = [max(2, int(v + (v & 1))) for v in vls]   # even widths
    offs = [0]
    for w in W:
        offs.append(offs[-1] + w)
    Wsum = offs[-1]
    nb = [max(1, math.ceil(v / 128)) for v in vls]
    nbmax = max(nb)
    vco = [0]
    for n_ in nb:
        vco.append(vco[-1] + n_)
    NVC = vco[-1]
    nDQ, nDK = DQ // 128, DK // 128
    QB = B * QG  # q columns per core
    NQD = len(QD_ORDER)
    NKT = len(KT_ORDER)

    nc = bass.Bass()
    qT_e = nc.declare_dram_parameter("qT", [128, nDQ, B, QG], BF16, isOutput=False)
    kT_e = nc.declare_dram_parameter("kT", [128, nDK, Wsum], BF16, isOutput=False)
    v_e = nc.declare_dram_parameter("v", [128, NVC, DV], BF16, isOutput=False)
    wq_e = nc.declare_dram_parameter("wq", [128, nDQ, H], BF16, isOutput=False)
    wk_e = nc.declare_dram_parameter("wk", [128, nDK, H], BF16, isOutput=False)
    wvc_e = nc.declare_dram_parameter("wvc", [128, NT], F32, isOutput=False)
    ones_e = nc.declare_dram_parameter("ones", [128, 1], BF16, isOutput=False)
    out_e = nc.declare_dram_parameter("out", [B, QG, DV], F32, isOutput=True)
    if debug:
        dbg_qz = nc.declare_dram_parameter("dbg_qz", [128, B, QG], BF16, isOutput=True)
        dbg_qd = nc.declare_dram_parameter("dbg_qd", [128, len(QD_ORDER), B * QG], BF16, isOutput=True)
        dbg_rhs = nc.declare_dram_parameter("dbg_rhs", [128, NT, B * QG], BF16, isOutput=True)
        dbg_ktl = nc.declare_dram_parameter("dbg_ktl", [128, len(KT_ORDER), Wsum], BF16, isOutput=True)
        dbg_exp = nc.declare_dram_parameter("dbg_exp", [128, 2, max(1, max(math.ceil(v / 128) for v in vls)), QG], BF16, isOutput=True)

    # ---- pass A: enumerate semaphore orders ------------------------------
    class S:
        def __init__(self):
            self.n = 0
            self.idx = {}

        def inc(self, tag=None):
            self.n += 1
            if tag is not None:
                self.idx[tag] = self.n
            return self.n

    pe, act, dve = S(), S(), S()
    LOADS = (["wq", "wk", "qT", "wvc", "ones"]
             + [f"kT{g}" for g in range(B)] + [f"v{g}" for g in range(B)])

    # terms on ACT-built tiles (z, z2, t) go first; rest second
    TS1 = [t for t in TERMS if KJ[SEL[t][1]] <= 2]
    TS2 = [t for t in TERMS if KJ[SEL[t][1]] > 2]

    for it in range(iters):
        for g in range(B):
            pe.inc(("qp", it, g))
        for g in range(B):
            pe.inc(("kp", it, g))
        for g in range(B):
            pe.inc(("sc", it, g))
            if g >= 1:
                pe.inc(("z", it, g - 1))
                pe.inc(("vm", it, g - 1))
        pe.inc(("z", it, B - 1))
        pe.inc(("vm", it, B - 1))

    for it in range(iters):
        act.inc(("tq", it))
        for g in range(B):
            act.inc(("kz", it, g))
            act.inc(("kz2", it, g))
            act.inc(("tk", it, g))
            if g >= 2:
                act.inc(("e", it, g - 2))
        act.inc(("e", it, B - 2))
        act.inc(("e", it, B - 1))

    dve.inc(("ms",))
    for it in range(iters):
        for g in range(B):
            dve.inc(("pc", it, g))
        dve.inc(("qd", it))
        for t in TS1:
            dve.inc(("ts", it, t))
        for j in range(3, NKT):
            dve.inc(("kt", it, 0, j))
        for t in TS2:
            dve.inc(("ts", it, t))
        for j in range(3, NKT):
            dve.inc(("kt", it, 1, j))
        for g in range(2, B):
            dve.inc(("r", it, g - 2))
            dve.inc(("o", it, g - 2))
            for j in range(3, NKT):
                dve.inc(("kt", it, g, j))
        dve.inc(("r", it, B - 2))
        dve.inc(("o", it, B - 2))
        dve.inc(("r", it, B - 1))
        dve.inc(("o", it, B - 1))

    # pp bank user sequence per iter: qp0..3, kp0..3 -> bank = u % 2
    def pp_bank(it, kind, g):
        u = it * (2 * B) + (g if kind == "qp" else B + g)
        return u % 2, u

    def pp_prior_reader(u):
        # reader of the output of pp-user (u-2); None if u < 2
        if u < 2:
            return None
        up = u - 2
        it, r = divmod(up, 2 * B)
        if r < B:
            return ("dve", ("pc", it, r))
        return ("act", ("tk", it, r - B))

    # ---- emit ------------------------------------------------------------
    from contextlib import ExitStack

    es = ExitStack()
    with es:
        wq_sb = es.enter_context(nc.sbuf_tensor([128, nDQ, H], BF16))
        wk_sb = es.enter_context(nc.sbuf_tensor([128, nDK, H], BF16))
        qT_sb = es.enter_context(nc.sbuf_tensor([128, nDQ, B, QG], BF16))
        kT_sb = es.enter_context(nc.sbuf_tensor([128, nDK, Wsum], BF16))
        v_sb = es.enter_context(nc.sbuf_tensor([128, NVC, DV], BF16))
        wvc_sb = es.enter_context(nc.sbuf_tensor([128, NT], F32))
        ones_sb = es.enter_context(nc.sbuf_tensor([128, 1], BF16))
        qz_sb = es.enter_context(nc.sbuf_tensor([128, B, QG], BF16))
        qd_sb = es.enter_context(nc.sbuf_tensor([128, NQD, QB], BF16))
        rhs_sb = es.enter_context(nc.sbuf_tensor([128, NT, QB], BF16))
        ktl_sb = es.enter_context(nc.sbuf_tensor([128, NKT, Wsum], BF16))
        exp_sb = es.enter_context(nc.sbuf_tensor([128, 2, nbmax, QG], BF16))
        recip_sb = es.enter_context(nc.sbuf_tensor([QG, B], F32))
        out_sb = es.enter_context(nc.sbuf_tensor([QG, 2, DV], F32))
        scratch = es.enter_context(nc.sbuf_tensor([1, 8], F32))
        scratch2 = es.enter_context(nc.sbuf_tensor([1, 8], F32))

        pp = [es.enter_context(nc.psum_tensor(f"pp{i}", [128, 512], F32))
              for i in range(2)]
        sc = [es.enter_context(nc.psum_tensor(f"sc{i}", [128, nbmax, QG], F32))
              for i in range(2)]
        op = [es.enter_context(nc.psum_tensor(f"op{i}", [QG, DV], F32))
              for i in range(2)]
        z_ps = es.enter_context(nc.psum_tensor("z_ps", [QG, B], F32))

        ld_sem = {name: es.enter_context(nc.semaphore(f"ld_{name}"))
                  for name in LOADS}
        ost_sem = [es.enter_context(nc.semaphore(f"ost{i}")) for i in range(2)]
        pe_sem = es.enter_context(nc.semaphore("pe_sem"))
        act_sem = es.enter_context(nc.semaphore("act_sem"))
        dve_sem = es.enter_context(nc.semaphore("dve_sem"))
        block = es.enter_context(nc.Block())

        # q-dict tile views: index in qd_sb by QD_ORDER position
        QDI = {qk: i for i, qk in enumerate(QD_ORDER)}

        @block.sync
        def _(sy):
            sy.dma_start(out=wq_sb[:], in_=wq_e[:]).then_inc(ld_sem["wq"], 16)
            sy.dma_start(out=qT_sb[:], in_=qT_e[:]).then_inc(ld_sem["qT"], 16)
            sy.dma_start(out=wk_sb[:], in_=wk_e[:]).then_inc(ld_sem["wk"], 16)
            sy.dma_start(out=wvc_sb[:], in_=wvc_e[:]).then_inc(ld_sem["wvc"], 16)
            for g in range(B):
                sy.dma_start(
                    out=kT_sb[:, :, offs[g]:offs[g] + W[g]],
                    in_=kT_e[:, :, offs[g]:offs[g] + W[g]],
                ).then_inc(ld_sem[f"kT{g}"], 16)
            sy.dma_start(out=ones_sb[:], in_=ones_e[:]).then_inc(ld_sem["ones"], 16)
            for g in range(B):
                sy.dma_start(
                    out=v_sb[:, vco[g]:vco[g] + nb[g], :],
                    in_=v_e[:, vco[g]:vco[g] + nb[g], :],
                ).then_inc(ld_sem[f"v{g}"], 16)
            for it in range(iters):
                for g in range(B):
                    gg = it * B + g
                    sy.wait_ge(dve_sem, dve.idx[("o", it, g)])
                    sy.dma_start(
                        out=out_e[g], in_=out_sb[0:QG, g % 2, :]
                    ).then_inc(ost_sem[g % 2], 16)
            if debug:
                sy.dma_start(out=dbg_qz[:], in_=qz_sb[:]).then_inc(ost_sem[0], 16)
                sy.dma_start(out=dbg_qd[:], in_=qd_sb[:]).then_inc(ost_sem[0], 16)
                sy.dma_start(out=dbg_rhs[:], in_=rhs_sb[:]).then_inc(ost_sem[0], 16)
                sy.dma_start(out=dbg_ktl[:], in_=ktl_sb[:]).then_inc(ost_sem[0], 16)
                sy.dma_start(out=dbg_exp[:], in_=exp_sb[:]).then_inc(ost_sem[0], 16)

        @block.tensor
        def _(pe_eng):
            def qp(it, g):
                bank, u = pp_bank(it, "qp", g)
                if it == 0 and g == 0:
                    pe_eng.wait_ge(ld_sem["wq"], 16)
                    pe_eng.wait_ge(ld_sem["qT"], 16)
                pr = pp_prior_reader(u)
                if pr is not None:
                    sem = dve_sem if pr[0] == "dve" else act_sem
                    idx = (dve if pr[0] == "dve" else act).idx[pr[1]]
                    pe_eng.wait_ge(sem, idx)
                for c in range(nDQ):
                    mm = pe_eng.matmul(
                        pp[bank][0:128, 0:QG], wq_sb[:, c, :],
                        qT_sb[:, c, g, :], start=(c == 0), stop=(c == nDQ - 1),
                    )
                mm.then_inc(pe_sem, 1)

            def kp(it, g):
                bank, u = pp_bank(it, "kp", g)
                if it == 0 and g == 0:
                    pe_eng.wait_ge(ld_sem["wk"], 16)
                if it == 0:
                    pe_eng.wait_ge(ld_sem[f"kT{g}"], 16)
                pr = pp_prior_reader(u)
                if pr is not None:
                    sem = dve_sem if pr[0] == "dve" else act_sem
                    idx = (dve if pr[0] == "dve" else act).idx[pr[1]]
                    pe_eng.wait_ge(sem, idx)
                for c in range(nDK):
                    mm = pe_eng.matmul(
                        pp[bank][0:128, 0:W[g]], wk_sb[:, c, :],
                        kT_sb[:, c, offs[g]:offs[g] + W[g]],
                        start=(c == 0), stop=(c == nDK - 1),
                    )
                mm.then_inc(pe_sem, 1)

            def scr(it, g):
                # psum slot reuse: previous user is exp(it', g-2)
                pg = it * B + g - 2
                if pg >= 0:
                    pe_eng.wait_ge(act_sem, act.idx[("e", pg // B, pg % B)])
                for b in range(nb[g]):
                    sz = min(128, vls[g] - 128 * b)
                    for ti, t in enumerate(TERMS):
                        qk, kn = SEL[t]
                        j = KJ[kn]
                        if b == 0:
                            pe_eng.wait_ge(dve_sem, dve.idx[("ts", it, t)])
                            if j == 0:
                                pe_eng.wait_ge(act_sem, act.idx[("kz", it, g)])
                            elif j == 1:
                                pe_eng.wait_ge(act_sem, act.idx[("kz2", it, g)])
                            elif j == 2:
                                pe_eng.wait_ge(act_sem, act.idx[("tk", it, g)])
                            else:
                                pe_eng.wait_ge(dve_sem, dve.idx[("kt", it, g, j)])
                        mm = pe_eng.matmul(
                            sc[g % 2][0:sz, b, :],
                            ktl_sb[:, j, offs[g] + 128 * b:offs[g] + 128 * b + sz],
                            rhs_sb[:, t, g * QG:(g + 1) * QG],
                            start=(ti == 0), stop=(ti == NT - 1),
                        )
                mm.then_inc(pe_sem, 1)

            def zmm(it, g):
                pe_eng.wait_ge(act_sem, act.idx[("e", it, g)])
                if it == 0 and g == 0:
                    pe_eng.wait_ge(ld_sem["ones"], 16)
                for b in range(nb[g]):
                    sz = min(128, vls[g] - 128 * b)
                    mm = pe_eng.matmul(
                        z_ps[0:QG, g:g + 1], exp_sb[0:sz, g % 2, b, :],
                        ones_sb[0:sz, :], start=(b == 0), stop=(b == nb[g] - 1),
                    )
                mm.then_inc(pe_sem, 1)

            def vmm(it, g):
                if it == 0:
                    pe_eng.wait_ge(ld_sem[f"v{g}"], 16)
                pg = it * B + g - 2
                if pg >= 0:
                    pe_eng.wait_ge(dve_sem, dve.idx[("o", pg // B, pg % B)])
                for b in range(nb[g]):
                    sz = min(128, vls[g] - 128 * b)
                    mm = pe_eng.matmul(
                        op[g % 2][0:QG, 0:DV], exp_sb[0:sz, g % 2, b, :],
                        v_sb[0:sz, vco[g] + b, :],
                        start=(b == 0), stop=(b == nb[g] - 1),
                    )
                mm.then_inc(pe_sem, 1)

            for it in range(iters):
                for g in range(B):
                    qp(it, g)
                for g in range(B):
                    kp(it, g)
                for g in range(B):
                    scr(it, g)
                    if g >= 1:
                        zmm(it, g - 1)
                        vmm(it, g - 1)
                zmm(it, B - 1)
                vmm(it, B - 1)

        @block.scalar
        def _(sa):
            sa.wait_ge(dve_sem, dve.idx[("ms",)])
            sa.activation(scratch2[0:1, 0:2], scratch[0:1, 0:2], AF.Tanh)

            def tq_op(it):
                sa.wait_ge(dve_sem, dve.idx[("pc", it, B - 1)])
                sa.activation(
                    qd_sb[:, QDI[(0, 1)], :],
                    qz_sb.rearrange("p b q -> p (b q)")[:, :],
                    AF.Tanh,
                ).then_inc(act_sem, 1)

            def k_ops(it, g):
                bank, u = pp_bank(it, "kp", g)
                sa.wait_ge(pe_sem, pe.idx[("kp", it, g)])
                sa.activation(
                    ktl_sb[:, 0, offs[g]:offs[g] + W[g]],
                    pp[bank][0:128, 0:W[g]], AF.Copy,
                ).then_inc(act_sem, 1)
                sa.activation(
                    ktl_sb[:, 1, offs[g]:offs[g] + W[g]],
                    pp[bank][0:128, 0:W[g]], AF.Square,
                ).then_inc(act_sem, 1)
                sa.activation(
                    ktl_sb[:, 2, offs[g]:offs[g] + W[g]],
                    pp[bank][0:128, 0:W[g]], AF.Tanh,
                ).then_inc(act_sem, 1)

            def e_op(it, g):
                sa.wait_ge(pe_sem, pe.idx[("sc", it, g)])
                sa.activation(
                    exp_sb[0:128, g % 2, 0:nb[g], :],
                    sc[g % 2][0:128, 0:nb[g], :], AF.Exp,
                ).then_inc(act_sem, 1)

            for it in range(iters):
                tq_op(it)
                for g in range(B):
                    k_ops(it, g)
                    if g >= 2:
                        e_op(it, g - 2)
                e_op(it, B - 2)
                e_op(it, B - 1)

        @block.vector
        def _(ve):
            ve.memset(scratch[0:1, 0:8], 0.0)
            ve.memset(sc[0][:], 0.0)
            ve.memset(sc[1][:], 0.0)
            ve.memset(qd_sb[:, QDI[(0, 0)], :], 1.0).then_inc(dve_sem, 1)

            qzv = qz_sb.rearrange("p b q -> p (b q)")

            def pc(it, g):
                bank, u = pp_bank(it, "qp", g)
                ve.wait_ge(pe_sem, pe.idx[("qp", it, g)])
                ve.tensor_copy(qz_sb[:, g, :], pp[bank][0:128, 0:QG]).then_inc(
                    dve_sem, 1)

            def qdict(it):
                q1 = qzv[:, :]
                q2 = qd_sb[:, QDI[(2, 0)], :]
                q3 = qd_sb[:, QDI[(3, 0)], :]
                tq = qd_sb[:, QDI[(0, 1)], :]
                ve.tensor_copy(qd_sb[:, QDI[(1, 0)], :], q1)
                ve.tensor_mul(q2, q1, q1)
                ve.tensor_mul(q3, q2, q1)
                ve.wait_ge(act_sem, act.idx[("tq", it)])
                ve.tensor_mul(qd_sb[:, QDI[(1, 1)], :], q1, tq)
                ve.tensor_mul(qd_sb[:, QDI[(2, 1)], :], q2, tq)
                ve.tensor_mul(qd_sb[:, QDI[(3, 1)], :], q3, tq).then_inc(
                    dve_sem, 1)

            def ts(it, t, first):
                if first and it == 0:
                    ve.wait_ge(ld_sem["wvc"], 16)
                qk = SEL[t][0]
                ve.tensor_scalar_mul(
                    rhs_sb[:, t, :], qd_sb[:, QDI[qk], :], wvc_sb[:, t:t + 1]
                ).then_inc(dve_sem, 1)

            def kt(it, g):
                z = ktl_sb[:, 0, offs[g]:offs[g] + W[g]]
                z2 = ktl_sb[:, 1, offs[g]:offs[g] + W[g]]
                tk = ktl_sb[:, 2, offs[g]:offs[g] + W[g]]
                z3 = ktl_sb[:, 3, offs[g]:offs[g] + W[g]]
                tz = ktl_sb[:, 4, offs[g]:offs[g] + W[g]]
                tz2 = ktl_sb[:, 5, offs[g]:offs[g] + W[g]]
                tz3 = ktl_sb[:, 6, offs[g]:offs[g] + W[g]]
                ve.wait_ge(act_sem, act.idx[("kz2", it, g)])
                ve.tensor_mul(z3, z, z2).then_inc(dve_sem, 1)
                ve.wait_ge(act_sem, act.idx[("tk", it, g)])
                ve.tensor_mul(tz, tk, z).then_inc(dve_sem, 1)
                ve.tensor_mul(tz2, tk, z2).then_inc(dve_sem, 1)
                ve.tensor_mul(tz3, tk, z3).then_inc(dve_sem, 1)

            def rec(it, g):
                ve.wait_ge(pe_sem, pe.idx[("z", it, g)])
                ve.reciprocal(recip_sb[0:QG, g:g + 1], z_ps[0:QG, g:g + 1]
                              ).then_inc(dve_sem, 1)

            def osc(it, g):
                gg = it * B + g
                ve.wait_ge(dve_sem, dve.idx[("r", it, g)])
                ve.wait_ge(pe_sem, pe.idx[("vm", it, g)])
                if gg >= 2:
                    ve.wait_ge(ost_sem[gg % 2], 16 * (gg // 2))
                ve.tensor_scalar_mul(
                    out_sb[0:QG, g % 2, :], op[g % 2][0:QG, 0:DV],
                    recip_sb[0:QG, g:g + 1],
                ).then_inc(dve_sem, 1)

            for it in range(iters):
                for g in range(B):
                    pc(it, g)
                qdict(it)
                for i, t in enumerate(TS1):
                    ts(it, t, i == 0)
                kt(it, 0)
                for t in TS2:
                    ts(it, t, False)
                kt(it, 1)
                for g in range(2, B):
                    rec(it, g - 2)
                    osc(it, g - 2)
                    kt(it, g)
                rec(it, B - 2)
                osc(it, B - 2)
                rec(it, B - 1)
                osc(it, B - 1)

    return nc


# ---------------------------------------------------------------------------
def _host_prep2(queries, keys, values, Wq, Wk, Wv, valid_lens,
                B, H, DQ, DK, DV, QG):
    bf = ml_dtypes.bfloat16
    vls = [int(v) for v in np.asarray(valid_lens)]
    W = [max(2, int(v + (v & 1))) for v in vls]
    offs = [0]
    for w in W:
        offs.append(offs[-1] + w)
    Wsum = offs[-1]
    nb = [max(1, math.ceil(v / 128)) for v in vls]
    vco = [0]
    for n_ in nb:
        vco.append(vco[-1] + n_)
    NVC = vco[-1]
    nDQ, nDK = DQ // 128, DK // 128

    qnp = np.asarray(queries, dtype=np.float32)
    knp = np.asarray(keys, dtype=np.float32)
    Wqn = np.asarray(Wq, dtype=np.float32)
    Wkn = np.asarray(Wk, dtype=np.float32)
    Wvn = np.asarray(Wv, dtype=np.float32)

    # input-scale estimates (no full projection needed)
    sq = float(np.sqrt((qnp**2).mean() * (Wqn**2).sum(0).mean()))
    sk = float(np.sqrt((knp**2).mean() * (Wkn**2).sum(0).mean()))
    coefs = fit_coefs(sq, sk)
    wvc = (Wvn.reshape(H, 1) * coefs.reshape(1, NT)).astype(np.float32)

    kT = np.zeros((DK, Wsum), np.float32)
    for b in range(B):
        kb = knp[b][:vls[b]].T
        kT[:, offs[b]:offs[b] + vls[b]] = kb
        if W[b] > vls[b]:
            kT[:, offs[b] + vls[b]] = kb[:, -1] if vls[b] else 0.0
    kT = kT.reshape(nDK, 128, Wsum).transpose(1, 0, 2)
    v = np.zeros((128 * NVC, DV), np.float32)
    for b in range(B):
        v[128 * vco[b]:128 * vco[b] + vls[b]] = values[b][:vls[b]]
    v = v.reshape(NVC, 128, DV).transpose(1, 0, 2)
    wq = (Wqn / sq).reshape(nDQ, 128, H).transpose(1, 0, 2)
    wk = (Wkn / sk).reshape(nDK, 128, H).transpose(1, 0, 2)
    qT_full = qnp.transpose(0, 2, 1).reshape(B, nDQ, 128, -1)
    qT_full = qT_full.transpose(2, 1, 0, 3)

    common = {
        "kT": np.ascontiguousarray(kT).astype(bf),
        "v": np.ascontiguousarray(v).astype(bf),
        "wq": np.ascontiguousarray(wq).astype(bf),
        "wk": np.ascontiguousarray(wk).astype(bf),
        "wvc": np.ascontiguousarray(wvc),
        "ones": np.ones((128, 1), dtype=bf),
    }
    in_maps = []
    for c in range(N_CORES):
        m = dict(common)
        m["qT"] = np.ascontiguousarray(
            qT_full[:, :, :, c * QG:(c + 1) * QG]).astype(bf)
        in_maps.append(m)
    return vls, in_maps


def kernel(queries, keys, values, Wq, Wk, Wv, valid_lens):
    B, NQ, DQ = queries.shape
    _, NK, DK = keys.shape
    DV = values.shape[2]
    H = Wq.shape[1]
    QG = NQ // N_CORES

    vls, in_maps = _host_prep2(
        queries, keys, values, Wq, Wk, Wv, valid_lens, B, H, DQ, DK, DV, QG)
    nc = build_graph2(vls, B=B, H=H, DQ=DQ, DK=DK, DV=DV, QG=QG)
    r = run_bass_kernel_spmd(nc, in_maps, core_ids=list(range(N_CORES)))
    out = np.empty((B, NQ, DV), np.float32)
    for c in range(N_CORES):
        out[:, c * QG:(c + 1) * QG, :] = r.results[c]["out"]
    return out


# revision 11
# speedup vs baseline: 2.5509x; 1.5848x over previous
"""Additive attention (Bahdanau) Trainium2 kernel, SPMD across 8 NeuronCores.

Reference computation (per batch b):
    q = Q[b] @ Wq                 [NQ, H]
    k = K[b] @ Wk                 [NK, H]
    scores[i, j] = sum_h Wv[h] * tanh(q[i, h] + k[j, h])
    attn = softmax(mask(scores))  (keys >= valid_len[b] masked to -1e6)
    out[b] = attn @ V[b]

KEY ALGORITHMIC CHANGE vs the tanh-materializing baseline: tanh(q+k) is a
smooth bivariate function of two ~N(0,1) scalars, so it admits a separable
(low-rank) approximation

    tanh(q + k) ~= sum_t c_t * d_{s(t)}(q) * K_{j(t)}(k)

with q-side dictionary {q^a * tanh(q)^e} and k-side menu
{z, z^2, z^3, t, t*z, t*z^2, t*z^3} (t = tanh(z)).  The structure (26
terms) is fixed; coefficients are re-fit at kernel() time by weighted
least squares on a Gauss-Hermite grid matched to the input scales.  The
[NQ, NK, H] intermediate is never materialized: scores become 26
PSUM-accumulated 128-contraction matmuls per (batch, key-block), with the
per-term coefficient and the Wv reduction folded into tiny [128,1]
per-partition scalars applied on the q side (tensor_scalar, DVE 4x mode).

Sharding: core c handles queries [c*QG, (c+1)*QG) of EVERY batch (QG =
NQ/8).  Each batch's key range is truncated to its valid_len at
graph-build time (valid_lens host-visible), so no masking is needed.
Softmax without max-subtraction (|scores| <~ 15, exp safe in f32).

Engine mapping (per core):
  PE   : q/k projections; 26-term score matmuls (full 128x128 array
         utilization); softmax-denominator ones-matmuls; attn @ V.
  ACT  : k-menu base tiles straight from the projection PSUM (Copy,
         Square, Tanh -- all in one act table with Exp: no table reload),
         q-side tanh, exp.
  DVE  : PSUM->SBUF q copies; dict/menu products (tensor_tensor, bf16
         2x); 26 scaled rhs copies (tensor_scalar, bf16 4x); reciprocal;
         1/Z output scaling.
  SYNC : all DMA.
"""

import math

import numpy as np
import ml_dtypes

import concourse.bass as bass
import concourse.mybir as mybir
from concourse.bass_utils import run_bass_kernel_spmd

BF16 = mybir.dt.bfloat16
F32 = mybir.dt.float32
AF = mybir.ActivationFunctionType

N_CORES = 8

# ---------------------------------------------------------------------------
# Approximation structure (fixed): backward-eliminated from the dense
# bilinear fit of tanh(sq*zq + sk*zk) over the product Gauss measure.
# q-dict keys: (a, e) -> zq^a * tanh(zq)^e ; k-menu names below.
KT_ORDER = ["z", "z2", "t", "z3", "tz", "tz2", "tz3"]
QD_ORDER = [(0, 0), (1, 0), (2, 0), (3, 0), (0, 1), (1, 1), (2, 1), (3, 1)]
SEL = [
    ((0, 0), "z"), ((0, 0), "z3"), ((0, 0), "tz2"),
    ((1, 0), "z2"), ((1, 0), "tz"), ((1, 0), "tz3"),
    ((2, 0), "z"), ((2, 0), "z3"), ((2, 0), "t"), ((2, 0), "tz2"),
    ((3, 0), "z2"), ((3, 0), "tz"),
    ((0, 1), "z2"), ((0, 1), "tz"), ((0, 1), "tz3"),
    ((1, 1), "z"), ((1, 1), "z3"), ((1, 1), "t"), ((1, 1), "tz2"),
    ((2, 1), "z2"), ((2, 1), "tz"), ((2, 1), "tz3"),
    ((3, 1), "z"), ((3, 1), "z3"), ((3, 1), "t"), ((3, 1), "tz2"),
]
NT = len(SEL)
# term emission order: grouped by k-tile build order so the PE can stream
KJ = {n: i for i, n in enumerate(KT_ORDER)}
TERMS = sorted(range(NT), key=lambda t: KJ[SEL[t][1]])  # term ids by tile


def _kfun(name, zk, tk):
    return {"z": zk, "z2": zk**2, "z3": zk**3, "t": tk,
            "tz": tk * zk, "tz2": tk * zk**2, "tz3": tk * zk**3,
            "c": np.ones_like(zk)}[name]


def fit_coefs(sq, sk, n=120):
    """Weighted LSQ coefficients for the fixed SEL structure, plus free
    constant-in-k pairs (softmax-invariant, dropped from the kernel).

    Ridge-regularized: the bf16 tiles round at ~0.4% relative, so a
    coefficient c on a term with L2(gauss) column norm ||A_t|| injects
    ~EPS*|c|*||A_t|| of incoherent score noise.  Choosing lambda to
    minimize  residual^2 + (EPS*||D c||)^2  trades fit error against
    bf16 noise amplification directly."""
    EPS = 0.004
    xs, wx = np.polynomial.hermite_e.hermegauss(n)
    wx = wx / wx.sum()
    zq = xs
    zk = xs
    F = np.tanh(sq * zq[:, None] + sk * zk[None, :])
    sw = np.sqrt(np.outer(wx, wx))
    tgt = (F * sw).ravel()
    tq = np.tanh(zq)
    tk = np.tanh(zk)
    qd = {(a, e): zq**a * tq**e for (a, e) in QD_ORDER}
    cols = []
    for (qk, kn) in SEL:
        cols.append((np.outer(qd[qk], _kfun(kn, zk, tk)) * sw).ravel())
    for qk in QD_ORDER:  # constant-in-k: free via softmax invariance
        cols.append((np.outer(qd[qk], _kfun("c", zk, tk)) * sw).ravel())
    A = np.stack(cols, axis=1)
    d = np.linalg.norm(A, axis=0)
    d[NT:] *= 1e-3  # const-pairs are noise-free (not emitted): barely penalize
    lam = 0.35     # empirically optimal vs bf16-simulated end-to-end error
    Ar = np.concatenate([A, lam * EPS * np.diag(d)], axis=0)
    br = np.concatenate([tgt, np.zeros(len(d))])
    coef, *_ = np.linalg.lstsq(Ar, br, rcond=None)
    return coef[:NT]


# ---------------------------------------------------------------------------
def build_graph2(vls, B=4, H=128, DQ=512, DK=512, DV=512, QG=64, iters=1,
                 debug=False):
    """Per-core bass graph. vls: per-batch valid lens (python ints).
    iters > 1 unrolls the whole per-core computation (everything except
    the one-time input loads) for marginal-cost timing."""
    assert H == 128 and DQ % 128 == 0 and DK % 128 == 0
    W = [max(2, int(v + (v & 1))) for v in vls]   # even widths
    offs = [0]
    for w in W:
        offs.append(offs[-1] + w)
    Wsum = offs[-1]
    nb = [max(1, math.ceil(v / 128)) for v in vls]
    nbmax = max(nb)
    vco = [0]
    for n_ in nb:
        vco.append(vco[-1] + n_)
    NVC = vco[-1]
    nDQ, nDK = DQ // 128, DK // 128
    QB = B * QG  # q columns per core
    NQD = len(QD_ORDER)
    NKT = len(KT_ORDER)

    nc = bass.Bass()
    qT_e = nc.declare_dram_parameter("qT", [128, nDQ, B, QG], BF16, isOutput=False)
    kT_e = nc.declare_dram_parameter("kT", [128, nDK, Wsum], BF16, isOutput=False)
    v_e = nc.declare_dram_parameter("v", [128, NVC, DV], BF16, isOutput=False)
    wq_e = nc.declare_dram_parameter("wq", [128, nDQ, H], BF16, isOutput=False)
    wk_e = nc.declare_dram_parameter("wk", [128, nDK, H], BF16, isOutput=False)
    wvc_e = nc.declare_dram_parameter("wvc", [128, NT], F32, isOutput=False)
    ones_e = nc.declare_dram_parameter("ones", [128, 1], BF16, isOutput=False)
    out_e = nc.declare_dram_parameter("out", [B, QG, DV], F32, isOutput=True)
    if debug:
        dbg_qz = nc.declare_dram_parameter("dbg_qz", [128, B, QG], BF16, isOutput=True)
        dbg_qd = nc.declare_dram_parameter("dbg_qd", [128, len(QD_ORDER), B * QG], BF16, isOutput=True)
        dbg_rhs = nc.declare_dram_parameter("dbg_rhs", [128, NT, B * QG], BF16, isOutput=True)
        dbg_ktl = nc.declare_dram_parameter("dbg_ktl", [128, len(KT_ORDER), Wsum], BF16, isOutput=True)
        dbg_exp = nc.declare_dram_parameter("dbg_exp", [128, 2, max(1, max(math.ceil(v / 128) for v in vls)), QG], BF16, isOutput=True)

    # ---- pass A: enumerate semaphore orders ------------------------------
    class S:
        def __init__(self):
            self.n = 0
            self.idx = {}

        def inc(self, tag=None):
            self.n += 1
            if tag is not None:
                self.idx[tag] = self.n
            return self.n

    pe, act, dve = S(), S(), S()
    LOADS = (["wq", "wk", "qT", "wvc", "ones"]
             + [f"kT{g}" for g in range(B)] + [f"v{g}" for g in range(B)])

    # terms on ACT-built tiles (z, z2, t) go first; rest second
    TS1 = [t for t in TERMS if KJ[SEL[t][1]] <= 2]
    TS2 = [t for t in TERMS if KJ[SEL[t][1]] > 2]

    for it in range(iters):
        for g in range(B):
            pe.inc(("qp", it, g))
        for g in range(B):
            pe.inc(("kp", it, g))
        for g in range(B):
            pe.inc(("sc", it, g))
            if g >= 1:
                pe.inc(("z", it, g - 1))
                pe.inc(("vm", it, g - 1))
        pe.inc(("z", it, B - 1))
        pe.inc(("vm", it, B - 1))

    for it in range(iters):
        act.inc(("tq", it))
        for g in range(B):
            act.inc(("kz", it, g))
            act.inc(("kz2", it, g))
            act.inc(("tk", it, g))
            if g >= 2:
                act.inc(("e", it, g - 2))
        act.inc(("e", it, B - 2))
        act.inc(("e", it, B - 1))

    dve.inc(("ms",))
    for it in range(iters):
        for g in range(B):
            dve.inc(("pc", it, g))
        dve.inc(("qd", it))
        for t in TS1:
            dve.inc(("ts", it, t))
        for j in range(3, NKT):
            dve.inc(("kt", it, 0, j))
        for t in TS2:
            dve.inc(("ts", it, t))
        for j in range(3, NKT):
            dve.inc(("kt", it, 1, j))
        for g in range(2, B):
            for j in range(3, NKT):
                dve.inc(("kt", it, g, j))
        for g in range(B):
            dve.inc(("r", it, g))
            dve.inc(("o", it, g))

    # pp bank user sequence per iter: qp0..3, kp0..3 -> bank = u % 2
    def pp_bank(it, kind, g):
        u = it * (2 * B) + (g if kind == "qp" else B + g)
        return u % 2, u

    def pp_prior_reader(u):
        # reader of the output of pp-user (u-2); None if u < 2
        if u < 2:
            return None
        up = u - 2
        it, r = divmod(up, 2 * B)
        if r < B:
            return ("dve", ("pc", it, r))
        return ("act", ("tk", it, r - B))

    class WCache:
        # skip redundant monotone semaphore waits (each wait is a SEQ instr)
        def __init__(self, eng):
            self.eng = eng
            self.seen = {}

        def __call__(self, sem, idx):
            if self.seen.get(id(sem), -1) < idx:
                self.eng.wait_ge(sem, idx)
                self.seen[id(sem)] = idx

    # ---- emit ------------------------------------------------------------
    from contextlib import ExitStack

    es = ExitStack()
    with es:
        wq_sb = es.enter_context(nc.sbuf_tensor([128, nDQ, H], BF16))
        wk_sb = es.enter_context(nc.sbuf_tensor([128, nDK, H], BF16))
        qT_sb = es.enter_context(nc.sbuf_tensor([128, nDQ, B, QG], BF16))
        kT_sb = es.enter_context(nc.sbuf_tensor([128, nDK, Wsum], BF16))
        v_sb = es.enter_context(nc.sbuf_tensor([128, NVC, DV], BF16))
        wvc_sb = es.enter_context(nc.sbuf_tensor([128, NT], F32))
        ones_sb = es.enter_context(nc.sbuf_tensor([128, 1], BF16))
        qz_sb = es.enter_context(nc.sbuf_tensor([128, B, QG], BF16))
        qd_sb = es.enter_context(nc.sbuf_tensor([128, NQD, QB], BF16))
        rhs_sb = es.enter_context(nc.sbuf_tensor([128, NT, QB], BF16))
        ktl_sb = es.enter_context(nc.sbuf_tensor([128, NKT, Wsum], BF16))
        exp_sb = es.enter_context(nc.sbuf_tensor([128, 2, nbmax, QG], BF16))
        recip_sb = es.enter_context(nc.sbuf_tensor([QG, B], F32))
        out_sb = es.enter_context(nc.sbuf_tensor([QG, 2, DV], F32))
        scratch = es.enter_context(nc.sbuf_tensor([1, 8], F32))
        scratch2 = es.enter_context(nc.sbuf_tensor([1, 8], F32))

        pp = [es.enter_context(nc.psum_tensor(f"pp{i}", [128, 512], F32))
              for i in range(2)]
        sc = [es.enter_context(nc.psum_tensor(f"sc{i}", [128, nbmax, QG], F32))
              for i in range(2)]
        op = [es.enter_context(nc.psum_tensor(f"op{i}", [QG, DV], F32))
              for i in range(2)]
        z_ps = es.enter_context(nc.psum_tensor("z_ps", [QG, B], F32))

        ld_sem = {name: es.enter_context(nc.semaphore(f"ld_{name}"))
                  for name in LOADS}
        ost_sem = [es.enter_context(nc.semaphore(f"ost{i}")) for i in range(2)]
        pe_sem = es.enter_context(nc.semaphore("pe_sem"))
        act_sem = es.enter_context(nc.semaphore("act_sem"))
        dve_sem = es.enter_context(nc.semaphore("dve_sem"))
        block = es.enter_context(nc.Block())

        # q-dict tile views: index in qd_sb by QD_ORDER position
        QDI = {qk: i for i, qk in enumerate(QD_ORDER)}

        @block.sync
        def _(sy):
            sy.dma_start(out=wq_sb[:], in_=wq_e[:]).then_inc(ld_sem["wq"], 16)
            sy.dma_start(out=qT_sb[:], in_=qT_e[:]).then_inc(ld_sem["qT"], 16)
            sy.dma_start(out=wk_sb[:], in_=wk_e[:]).then_inc(ld_sem["wk"], 16)
            sy.dma_start(out=wvc_sb[:], in_=wvc_e[:]).then_inc(ld_sem["wvc"], 16)
            for g in range(B):
                sy.dma_start(
                    out=kT_sb[:, :, offs[g]:offs[g] + W[g]],
                    in_=kT_e[:, :, offs[g]:offs[g] + W[g]],
                ).then_inc(ld_sem[f"kT{g}"], 16)
            sy.dma_start(out=ones_sb[:], in_=ones_e[:]).then_inc(ld_sem["ones"], 16)
            for g in range(B):
                sy.dma_start(
                    out=v_sb[:, vco[g]:vco[g] + nb[g], :],
                    in_=v_e[:, vco[g]:vco[g] + nb[g], :],
                ).then_inc(ld_sem[f"v{g}"], 16)
            for it in range(iters):
                for g in range(B):
                    gg = it * B + g
                    sy.wait_ge(dve_sem, dve.idx[("o", it, g)])
                    sy.dma_start(
                        out=out_e[g], in_=out_sb[0:QG, g % 2, :]
                    ).then_inc(ost_sem[g % 2], 16)
            if debug:
                sy.dma_start(out=dbg_qz[:], in_=qz_sb[:]).then_inc(ost_sem[0], 16)
                sy.dma_start(out=dbg_qd[:], in_=qd_sb[:]).then_inc(ost_sem[0], 16)
                sy.dma_start(out=dbg_rhs[:], in_=rhs_sb[:]).then_inc(ost_sem[0], 16)
                sy.dma_start(out=dbg_ktl[:], in_=ktl_sb[:]).then_inc(ost_sem[0], 16)
                sy.dma_start(out=dbg_exp[:], in_=exp_sb[:]).then_inc(ost_sem[0], 16)

        @block.tensor
        def _(pe_eng):
            pw = WCache(pe_eng)
            def qp(it, g):
                bank, u = pp_bank(it, "qp", g)
                if it == 0 and g == 0:
                    pw(ld_sem["wq"], 16)
                    pw(ld_sem["qT"], 16)
                pr = pp_prior_reader(u)
                if pr is not None:
                    sem = dve_sem if pr[0] == "dve" else act_sem
                    idx = (dve if pr[0] == "dve" else act).idx[pr[1]]
                    pw(sem, idx)
                for c in range(nDQ):
                    mm = pe_eng.matmul(
                        pp[bank][0:128, 0:QG], wq_sb[:, c, :],
                        qT_sb[:, c, g, :], start=(c == 0), stop=(c == nDQ - 1),
                    )
                mm.then_inc(pe_sem, 1)

            def kp(it, g):
                bank, u = pp_bank(it, "kp", g)
                if it == 0 and g == 0:
                    pw(ld_sem["wk"], 16)
                if it == 0:
                    pw(ld_sem[f"kT{g}"], 16)
                pr = pp_prior_reader(u)
                if pr is not None:
                    sem = dve_sem if pr[0] == "dve" else act_sem
                    idx = (dve if pr[0] == "dve" else act).idx[pr[1]]
                    pw(sem, idx)
                for c in range(nDK):
                    mm = pe_eng.matmul(
                        pp[bank][0:128, 0:W[g]], wk_sb[:, c, :],
                        kT_sb[:, c, offs[g]:offs[g] + W[g]],
                        start=(c == 0), stop=(c == nDK - 1),
                    )
                mm.then_inc(pe_sem, 1)

            def scr(it, g):
                # psum slot reuse: previous user is exp(it', g-2)
                pg = it * B + g - 2
                if pg >= 0:
                    pw(act_sem, act.idx[("e", pg // B, pg % B)])
                for b in range(nb[g]):
                    sz = min(128, vls[g] - 128 * b)
                    for ti, t in enumerate(TERMS):
                        qk, kn = SEL[t]
                        j = KJ[kn]
                        if b == 0:
                            pw(dve_sem, dve.idx[("ts", it, t)])
                            if j == 0:
                                pw(act_sem, act.idx[("kz", it, g)])
                            elif j == 1:
                                pw(act_sem, act.idx[("kz2", it, g)])
                            elif j == 2:
                                pw(act_sem, act.idx[("tk", it, g)])
                            else:
                                pw(dve_sem, dve.idx[("kt", it, g, j)])
                        mm = pe_eng.matmul(
                            sc[g % 2][0:sz, b, :],
                            ktl_sb[:, j, offs[g] + 128 * b:offs[g] + 128 * b + sz],
                            rhs_sb[:, t, g * QG:(g + 1) * QG],
                            start=(ti == 0), stop=(ti == NT - 1),
                        )
                mm.then_inc(pe_sem, 1)

            def zmm(it, g):
                pw(act_sem, act.idx[("e", it, g)])
                if it == 0 and g == 0:
                    pw(ld_sem["ones"], 16)
                for b in range(nb[g]):
                    sz = min(128, vls[g] - 128 * b)
                    mm = pe_eng.matmul(
                        z_ps[0:QG, g:g + 1], exp_sb[0:sz, g % 2, b, :],
                        ones_sb[0:sz, :], start=(b == 0), stop=(b == nb[g] - 1),
                    )
                mm.then_inc(pe_sem, 1)

            def vmm(it, g):
                if it == 0:
                    pw(ld_sem[f"v{g}"], 16)
                pg = it * B + g - 2
                if pg >= 0:
                    pw(dve_sem, dve.idx[("o", pg // B, pg % B)])
                for b in range(nb[g]):
                    sz = min(128, vls[g] - 128 * b)
                    mm = pe_eng.matmul(
                        op[g % 2][0:QG, 0:DV], exp_sb[0:sz, g % 2, b, :],
                        v_sb[0:sz, vco[g] + b, :],
                        start=(b == 0), stop=(b == nb[g] - 1),
                    )
                mm.then_inc(pe_sem, 1)

            for it in range(iters):
                for g in range(B):
                    qp(it, g)
                for g in range(B):
                    kp(it, g)
                for g in range(B):
                    scr(it, g)
                    if g >= 1:
                        zmm(it, g - 1)
                        vmm(it, g - 1)
                zmm(it, B - 1)
                vmm(it, B - 1)

        @block.scalar
        def _(sa):
            aw = WCache(sa)
            aw(dve_sem, dve.idx[("ms",)])
            sa.activation(scratch2[0:1, 0:2], scratch[0:1, 0:2], AF.Tanh)

            def tq_op(it):
                aw(dve_sem, dve.idx[("pc", it, B - 1)])
                sa.activation(
                    qd_sb[:, QDI[(0, 1)], :],
                    qz_sb.rearrange("p b q -> p (b q)")[:, :],
                    AF.Tanh,
                ).then_inc(act_sem, 1)

            def k_ops(it, g):
                bank, u = pp_bank(it, "kp", g)
                aw(pe_sem, pe.idx[("kp", it, g)])
                sa.activation(
                    ktl_sb[:, 0, offs[g]:offs[g] + W[g]],
                    pp[bank][0:128, 0:W[g]], AF.Copy,
                ).then_inc(act_sem, 1)
                sa.activation(
                    ktl_sb[:, 1, offs[g]:offs[g] + W[g]],
                    pp[bank][0:128, 0:W[g]], AF.Square,
                ).then_inc(act_sem, 1)
                sa.activation(
                    ktl_sb[:, 2, offs[g]:offs[g] + W[g]],
                    pp[bank][0:128, 0:W[g]], AF.Tanh,
                ).then_inc(act_sem, 1)

            def e_op(it, g):
                aw(pe_sem, pe.idx[("sc", it, g)])
                sa.activation(
                    exp_sb[0:128, g % 2, 0:nb[g], :],
                    sc[g % 2][0:128, 0:nb[g], :], AF.Exp,
                ).then_inc(act_sem, 1)

            for it in range(iters):
                tq_op(it)
                for g in range(B):
                    k_ops(it, g)
                    if g >= 2:
                        e_op(it, g - 2)
                e_op(it, B - 2)
                e_op(it, B - 1)

        @block.vector
        def _(ve):
            vw = WCache(ve)
            ve.memset(scratch[0:1, 0:8], 0.0)
            ve.memset(sc[0][:], 0.0)
            ve.memset(sc[1][:], 0.0)
            ve.memset(qd_sb[:, QDI[(0, 0)], :], 1.0).then_inc(dve_sem, 1)

            qzv = qz_sb.rearrange("p b q -> p (b q)")

            def pc(it, g):
                bank, u = pp_bank(it, "qp", g)
                vw(pe_sem, pe.idx[("qp", it, g)])
                ve.tensor_copy(qz_sb[:, g, :], pp[bank][0:128, 0:QG]).then_inc(
                    dve_sem, 1)

            def qdict(it):
                q1 = qzv[:, :]
                q2 = qd_sb[:, QDI[(2, 0)], :]
                q3 = qd_sb[:, QDI[(3, 0)], :]
                tq = qd_sb[:, QDI[(0, 1)], :]
                ve.tensor_copy(qd_sb[:, QDI[(1, 0)], :], q1)
                ve.tensor_mul(q2, q1, q1)
                ve.tensor_mul(q3, q2, q1)
                vw(act_sem, act.idx[("tq", it)])
                ve.tensor_mul(qd_sb[:, QDI[(1, 1)], :], q1, tq)
                ve.tensor_mul(qd_sb[:, QDI[(2, 1)], :], q2, tq)
                ve.tensor_mul(qd_sb[:, QDI[(3, 1)], :], q3, tq).then_inc(
                    dve_sem, 1)

            def ts(it, t, first):
                if first and it == 0:
                    vw(ld_sem["wvc"], 16)
                qk = SEL[t][0]
                ve.tensor_scalar_mul(
                    rhs_sb[:, t, :], qd_sb[:, QDI[qk], :], wvc_sb[:, t:t + 1]
                ).then_inc(dve_sem, 1)

            def kt(it, g):
                z = ktl_sb[:, 0, offs[g]:offs[g] + W[g]]
                z2 = ktl_sb[:, 1, offs[g]:offs[g] + W[g]]
                tk = ktl_sb[:, 2, offs[g]:offs[g] + W[g]]
                z3 = ktl_sb[:, 3, offs[g]:offs[g] + W[g]]
                tz = ktl_sb[:, 4, offs[g]:offs[g] + W[g]]
                tz2 = ktl_sb[:, 5, offs[g]:offs[g] + W[g]]
                tz3 = ktl_sb[:, 6, offs[g]:offs[g] + W[g]]
                vw(act_sem, act.idx[("kz2", it, g)])
                ve.tensor_mul(z3, z, z2).then_inc(dve_sem, 1)
                vw(act_sem, act.idx[("tk", it, g)])
                ve.tensor_mul(tz, tk, z).then_inc(dve_sem, 1)
                ve.tensor_mul(tz2, tk, z2).then_inc(dve_sem, 1)
                ve.tensor_mul(tz3, tk, z3).then_inc(dve_sem, 1)

            def rec(it, g):
                vw(pe_sem, pe.idx[("z", it, g)])
                ve.reciprocal(recip_sb[0:QG, g:g + 1], z_ps[0:QG, g:g + 1]
                              ).then_inc(dve_sem, 1)

            def osc(it, g):
                gg = it * B + g
                vw(dve_sem, dve.idx[("r", it, g)])
                vw(pe_sem, pe.idx[("vm", it, g)])
                if gg >= 2:
                    vw(ost_sem[gg % 2], 16 * (gg // 2))
                ve.tensor_scalar_mul(
                    out_sb[0:QG, g % 2, :], op[g % 2][0:QG, 0:DV],
                    recip_sb[0:QG, g:g + 1],
                ).then_inc(dve_sem, 1)

            for it in range(iters):
                for g in range(B):
                    pc(it, g)
                qdict(it)
                for i, t in enumerate(TS1):
                    ts(it, t, i == 0)
                kt(it, 0)
                for t in TS2:
                    ts(it, t, False)
                kt(it, 1)
                for g in range(2, B):
                    kt(it, g)
                for g in range(B):
                    rec(it, g)
                    osc(it, g)

    return nc


# ---------------------------------------------------------------------------
def _host_prep2(queries, keys, values, Wq, Wk, Wv, valid_lens,
                B, H, DQ, DK, DV, QG):
    bf = ml_dtypes.bfloat16
    vls = [int(v) for v in np.asarray(valid_lens)]
    W = [max(2, int(v + (v & 1))) for v in vls]
    offs = [0]
    for w in W:
        offs.append(offs[-1] + w)
    Wsum = offs[-1]
    nb = [max(1, math.ceil(v / 128)) for v in vls]
    vco = [0]
    for n_ in nb:
        vco.append(vco[-1] + n_)
    NVC = vco[-1]
    nDQ, nDK = DQ // 128, DK // 128

    qnp = np.asarray(queries, dtype=np.float32)
    knp = np.asarray(keys, dtype=np.float32)
    Wqn = np.asarray(Wq, dtype=np.float32)
    Wkn = np.asarray(Wk, dtype=np.float32)
    Wvn = np.asarray(Wv, dtype=np.float32)

    # input-scale estimates (no full projection needed)
    sq = float(np.sqrt((qnp**2).mean() * (Wqn**2).sum(0).mean()))
    sk = float(np.sqrt((knp**2).mean() * (Wkn**2).sum(0).mean()))
    coefs = fit_coefs(sq, sk)
    wvc = (Wvn.reshape(H, 1) * coefs.reshape(1, NT)).astype(np.float32)

    kT = np.zeros((DK, Wsum), np.float32)
    for b in range(B):
        kb = knp[b][:vls[b]].T
        kT[:, offs[b]:offs[b] + vls[b]] = kb
        if W[b] > vls[b]:
            kT[:, offs[b] + vls[b]] = kb[:, -1] if vls[b] else 0.0
    kT = kT.reshape(nDK, 128, Wsum).transpose(1, 0, 2)
    v = np.zeros((128 * NVC, DV), np.float32)
    for b in range(B):
        v[128 * vco[b]:128 * vco[b] + vls[b]] = values[b][:vls[b]]
    v = v.reshape(NVC, 128, DV).transpose(1, 0, 2)
    wq = (Wqn / sq).reshape(nDQ, 128, H).transpose(1, 0, 2)
    wk = (Wkn / sk).reshape(nDK, 128, H).transpose(1, 0, 2)
    qT_full = qnp.transpose(0, 2, 1).reshape(B, nDQ, 128, -1)
    qT_full = qT_full.transpose(2, 1, 0, 3)

    common = {
        "kT": np.ascontiguousarray(kT).astype(bf),
        "v": np.ascontiguousarray(v).astype(bf),
        "wq": np.ascontiguousarray(wq).astype(bf),
        "wk": np.ascontiguousarray(wk).astype(bf),
        "wvc": np.ascontiguousarray(wvc),
        "ones": np.ones((128, 1), dtype=bf),
    }
    in_maps = []
    for c in range(N_CORES):
        m = dict(common)
        m["qT"] = np.ascontiguousarray(
            qT_full[:, :, :, c * QG:(c + 1) * QG]).astype(bf)
        in_maps.append(m)
    return vls, in_maps


def kernel(queries, keys, values, Wq, Wk, Wv, valid_lens):
    B, NQ, DQ = queries.shape
    _, NK, DK = keys.shape
    DV = values.shape[2]
    H = Wq.shape[1]
    QG = NQ // N_CORES

    vls, in_maps = _host_prep2(
        queries, keys, values, Wq, Wk, Wv, valid_lens, B, H, DQ, DK, DV, QG)
    nc = build_graph2(vls, B=B, H=H, DQ=DQ, DK=DK, DV=DV, QG=QG)
    r = run_bass_kernel_spmd(nc, in_maps, core_ids=list(range(N_CORES)))
    out = np.empty((B, NQ, DV), np.float32)
    for c in range(N_CORES):
        out[:, c * QG:(c + 1) * QG, :] = r.results[c]["out"]
    return out
